# revision 20
# baseline (speedup 1.0000x reference)
"""MoE gating network (logits -> softmax -> top-2) on 8 trn2 NeuronCores.

Reference computation (jax):
    logits = einsum("bsd,ed->bse", x, gate_w) + gate_b     # [4, 4096, 64]
    weights = softmax(logits, axis=-1)
    topk_weights, topk_indices = top_k(weights, 2)
    return topk_weights, topk_indices, weights

Sharding: data parallel over tokens. 16384 tokens split into 8 shards of
2048; the tiny gate weight is replicated. Each core computes its shard's
logits on the PE (f32), softmax on ACT/DVE, and top-2 via the DVE
InstMax/InstMaxIndex ops (8 largest per partition, descending).

Per-core pipeline, 16 blocks of 128 tokens:
- x is host-packed to [block, partition, k-chunk, token] so each block's
  1 MB load is 128 descriptors of 8 KB (d_model lands on partitions, as
  the PE contraction requires, without any on-chip transpose).
- gate_b is folded into the PSUM accumulation as a K=1 matmul of a ones
  row against the bias row, so the ACT exp reads logits straight from
  PSUM (accum_out gives the softmax denominator in the same pass).
- top-2 values and indices are compacted per tile into one packed f32
  tile (indices as exact small floats) and stored once at the end.
- outputs use partition-major device layouts, unpermuted on the host.
"""

import numpy as np

import concourse.mybir as mybir
import concourse.tile as tile
from concourse import bacc
from concourse.bass_utils import run_bass_kernel_spmd

# Problem shape (hardcoded per contract; kernel.py must be self-contained).
B, S, D, E = 4, 4096, 2048, 64
NCORES = 8
TOK = B * S                  # 16384 tokens
TPC = TOK // NCORES          # 2048 tokens per core
P = 128                      # partitions
KC = D // P                  # 16 contraction chunks
NMT = TPC // P               # 16 token-tiles (= blocks) per core
F32 = mybir.dt.float32
U32 = mybir.dt.uint32

_cache = {}


def _build_program():
    nc = bacc.Bacc(
        "TRN2", target_bir_lowering=False, debug=False, num_devices=NCORES
    )

    xt = nc.dram_tensor("xt", [NMT, P, KC, P], F32, kind="ExternalInput").ap()
    wtp = nc.dram_tensor("wtp", [P, KC * E], F32, kind="ExternalInput").ap()
    gb = nc.dram_tensor("gb", [1, E], F32, kind="ExternalInput").ap()
    # Outputs in partition-major layouts; host unpermutes.
    w_out = nc.dram_tensor("w_out", [P, NMT, E], F32, kind="ExternalOutput").ap()
    # Packed top-2: [p, j, 0:2] = values, [p, j, 2:4] = indices (as floats).
    tvi_out = nc.dram_tensor("tvi_out", [P, NMT * 4], F32, kind="ExternalOutput").ap()

    with tile.TileContext(nc) as tc:
        with (
            tc.tile_pool(name="const", bufs=1) as const_pool,
            tc.tile_pool(name="xin", bufs=8) as x_pool,
            tc.tile_pool(name="psum", bufs=8, space="PSUM") as psum_pool,
            tc.tile_pool(name="sm", bufs=6) as sm_pool,
            tc.tile_pool(name="stat", bufs=8) as stat_pool,
            tc.tile_pool(name="wout", bufs=3) as wout_pool,
            tc.tile_pool(name="top", bufs=4) as top_pool,
            tc.tile_pool(name="tvi", bufs=1) as tvi_pool,
        ):
            wt_sb = const_pool.tile([P, KC, E], F32)
            nc.sync.dma_start(out=wt_sb[:], in_=wtp.rearrange("p (c e) -> p c e", c=KC))
            gb_sb = const_pool.tile([1, E], F32)
            nc.sync.dma_start(out=gb_sb[:], in_=gb[:])
            ones_sb = const_pool.tile([1, P], F32)
            nc.vector.memset(ones_sb[:], 1.0)

            tvi_t = tvi_pool.tile([P, NMT, 4], F32)

            w_pair = None
            for j in range(NMT):
                x_t = x_pool.tile([P, KC, P], F32)
                # Quarter-loads: each 4-chunk group of contraction matmuls
                # only waits on its own quarter, so the PE trails the DMA
                # stream by ~1 quarter instead of a whole tile. (Finer
                # splits lose: the exclusive HWDGE device costs ~632 ns per
                # dma_start and becomes the bottleneck past ~70 DMAs; the
                # final tile uses halves — fewer fixed ~300 ns per-group
                # PE sem stalls on the critical tail.)
                nsplit = 2 if j == NMT - 1 else 4
                step = KC // nsplit
                for q in range(nsplit):
                    nc.sync.dma_start(
                        out=x_t[:, step * q:step * (q + 1), :],
                        in_=xt[j, :, step * q:step * (q + 1), :],
                    )
                if j % 2 == 0:
                    w_pair = wout_pool.tile([P, 2, E], F32)
                ps = psum_pool.tile([P, E], F32)
                # Bias row seeds the accumulation: ps = ones.T @ gate_b.
                nc.tensor.matmul(ps[:], ones_sb[:], gb_sb[:], start=True, stop=False)
                for c in range(KC):
                    nc.tensor.matmul(
                        ps[:],
                        x_t[:, c, :],
                        wt_sb[:, c, :],
                        start=False,
                        stop=(c == KC - 1),
                    )
                e_t = sm_pool.tile([P, E], F32, tag="exp")
                ssum = stat_pool.tile([P, 1], F32, tag="sum")
                nc.scalar.activation(
                    e_t[:], ps[:],
                    mybir.ActivationFunctionType.Exp,
                    accum_out=ssum[:],
                )
                rcp = stat_pool.tile([P, 1], F32, tag="rcp")
                nc.vector.reciprocal(rcp[:], ssum[:])
                nc.vector.tensor_scalar_mul(w_pair[:, j % 2, :], e_t[:], rcp[:])
                # Top-2 on the unnormalized exps (same order as the softmax
                # weights: positive per-token scale) — runs without waiting
                # for the normalize; values then get the same e*rcp product.
                vals8 = top_pool.tile([P, 8], F32, tag="vals")
                idx8 = top_pool.tile([P, 8], U32, tag="idx")
                nc.vector.max(vals8[:], e_t[:])
                nc.vector.max_index(idx8[:], vals8[:], e_t[:])
                nc.vector.tensor_scalar_mul(tvi_t[:, j, 0:2], vals8[:, 0:2], rcp[:])
                nc.vector.tensor_copy(tvi_t[:, j, 2:4], idx8[:, 0:2])
                if j % 2 == 1:
                    # Mid-stream stores go through the otherwise-idle GpSimd
                    # SWDGE ring: ACT's FIFO would head-of-line block later
                    # exp dispatches, SP's would delay loads.
                    nc.gpsimd.dma_start(
                        out=w_out[:, j - 1:j + 1, :], in_=w_pair[:]
                    )
                    if j == NMT - 5:
                        # Tiles 0..11 of the packed top-2 are final by now;
                        # store them mid-stream so only the last 4 tiles'
                        # slice has to land after the compute tail.
                        nc.gpsimd.dma_start(
                            out=tvi_out.rearrange(
                                "p (j k) -> p j k", j=NMT
                            )[:, :NMT - 4, :],
                            in_=tvi_t[:, :NMT - 4, :],
                        )

            # SP's ring is idle by now; runs parallel to the last w store.
            nc.sync.dma_start(
                out=tvi_out.rearrange("p (j k) -> p j k", j=NMT)[:, NMT - 4:, :],
                in_=tvi_t[:, NMT - 4:, :],
            )

    nc.compile()
    return nc


def _get_program():
    if "nc" not in _cache:
        _cache["nc"] = _build_program()
    return _cache["nc"]


def _prep_inputs(x, gate_w, gate_b):
    x2d = np.ascontiguousarray(x, dtype=np.float32).reshape(TOK, D)
    # Pack gate weight: wtp[p, c, e] = gate_w[e, c*128 + p]
    wtp = np.ascontiguousarray(
        gate_w.T.reshape(KC, P, E).transpose(1, 0, 2)
    ).reshape(P, KC * E)
    gb = np.ascontiguousarray(gate_b, dtype=np.float32).reshape(1, E)
    in_maps = []
    for s in range(NCORES):
        sh = x2d[s * TPC:(s + 1) * TPC]
        # xp[j, p, c, t] = x[j*128 + t, c*128 + p]
        xp = np.ascontiguousarray(
            sh.reshape(NMT, P, KC, P).transpose(0, 3, 2, 1)
        )
        in_maps.append({"xt": xp, "wtp": wtp, "gb": gb})
    return in_maps


def kernel(x, gate_w, gate_b, _trace=False, _trace_kwargs=None):
    nc = _get_program()
    in_maps = _prep_inputs(x, gate_w, gate_b)
    res = run_bass_kernel_spmd(
        nc, in_maps, list(range(NCORES)), trace=_trace,
        **(_trace_kwargs or {}),
    )
    outs = res.results

    w_shards, tv_shards, ti_shards = [], [], []
    for s in range(NCORES):
        wdev = outs[s]["w_out"].reshape(P, NMT, E)
        w_shards.append(wdev.transpose(1, 0, 2).reshape(TPC, E))
        tvi = outs[s]["tvi_out"].reshape(P, NMT, 4)
        tv_shards.append(tvi[:, :, 0:2].transpose(1, 0, 2).reshape(TPC, 2))
        ti_shards.append(tvi[:, :, 2:4].transpose(1, 0, 2).reshape(TPC, 2))

    topk_w = np.concatenate(tv_shards, axis=0)
    topk_i = np.concatenate(ti_shards, axis=0)
    weights = np.concatenate(w_shards, axis=0)
    out = (
        topk_w.reshape(B, S, 2).astype(np.float32),
        np.rint(topk_i).reshape(B, S, 2).astype(np.int32),
        weights.reshape(B, S, E).astype(np.float32),
    )
    if _trace:
        return out, res
    return out


# revision 21
# speedup vs baseline: 1.0075x; 1.0075x over previous
"""MoE gating network (logits -> softmax -> top-2) on 8 trn2 NeuronCores.

Reference computation (jax):
    logits = einsum("bsd,ed->bse", x, gate_w) + gate_b     # [4, 4096, 64]
    weights = softmax(logits, axis=-1)
    topk_weights, topk_indices = top_k(weights, 2)
    return topk_weights, topk_indices, weights

Sharding: data parallel over tokens. 16384 tokens split into 8 shards of
2048; the tiny gate weight is replicated. Each core computes its shard's
logits on the PE (f32), softmax on ACT/DVE, and top-2 via the DVE
InstMax/InstMaxIndex ops (8 largest per partition, descending).

Per-core pipeline, 16 blocks of 128 tokens:
- x is host-packed to [block, partition, k-chunk, token] so each block's
  1 MB load is 128 descriptors of 8 KB (d_model lands on partitions, as
  the PE contraction requires, without any on-chip transpose).
- gate_b is folded into the PSUM accumulation as a K=1 matmul of a ones
  row against the bias row, so the ACT exp reads logits straight from
  PSUM (accum_out gives the softmax denominator in the same pass).
- top-2 values and indices are compacted per tile into one packed f32
  tile (indices as exact small floats) and stored once at the end.
- outputs use partition-major device layouts, unpermuted on the host.
"""

import numpy as np

import concourse.mybir as mybir
import concourse.tile as tile
from concourse import bacc
from concourse.bass_utils import run_bass_kernel_spmd

# Problem shape (hardcoded per contract; kernel.py must be self-contained).
B, S, D, E = 4, 4096, 2048, 64
NCORES = 8
TOK = B * S                  # 16384 tokens
TPC = TOK // NCORES          # 2048 tokens per core
P = 128                      # partitions
KC = D // P                  # 16 contraction chunks
NMT = TPC // P               # 16 token-tiles (= blocks) per core
F32 = mybir.dt.float32
U32 = mybir.dt.uint32

_cache = {}


def _build_program():
    nc = bacc.Bacc(
        "TRN2", target_bir_lowering=False, debug=False, num_devices=NCORES
    )

    xt = nc.dram_tensor("xt", [NMT, P, KC, P], F32, kind="ExternalInput").ap()
    wtp = nc.dram_tensor("wtp", [P, KC * E], F32, kind="ExternalInput").ap()
    gb = nc.dram_tensor("gb", [1, E], F32, kind="ExternalInput").ap()
    # Outputs in partition-major layouts; host unpermutes.
    w_out = nc.dram_tensor("w_out", [P, NMT, E], F32, kind="ExternalOutput").ap()
    # Packed top-2: [p, j, 0:2] = values, [p, j, 2:4] = indices (as floats).
    tvi_out = nc.dram_tensor("tvi_out", [P, NMT * 4], F32, kind="ExternalOutput").ap()

    with tile.TileContext(nc) as tc:
        with (
            tc.tile_pool(name="const", bufs=1) as const_pool,
            tc.tile_pool(name="xin", bufs=8) as x_pool,
            tc.tile_pool(name="psum", bufs=8, space="PSUM") as psum_pool,
            tc.tile_pool(name="sm", bufs=6) as sm_pool,
            tc.tile_pool(name="stat", bufs=8) as stat_pool,
            tc.tile_pool(name="wout", bufs=3) as wout_pool,
            tc.tile_pool(name="top", bufs=4) as top_pool,
            tc.tile_pool(name="tvi", bufs=1) as tvi_pool,
        ):
            wt_sb = const_pool.tile([P, KC, E], F32)
            nc.sync.dma_start(out=wt_sb[:], in_=wtp.rearrange("p (c e) -> p c e", c=KC))
            gb_sb = const_pool.tile([1, E], F32)
            nc.sync.dma_start(out=gb_sb[:], in_=gb[:])
            ones_sb = const_pool.tile([1, P], F32)
            nc.vector.memset(ones_sb[:], 1.0)

            tvi_t = tvi_pool.tile([P, NMT, 4], F32)

            w_pair = None
            for j in range(NMT):
                x_t = x_pool.tile([P, KC, P], F32)
                # Quarter-loads: each 4-chunk group of contraction matmuls
                # only waits on its own quarter, so the PE trails the DMA
                # stream by ~1 quarter instead of a whole tile. (Both finer
                # and coarser splits measured slower: finer saturates the
                # exclusive HWDGE device at ~632 ns per dma_start; coarser
                # lengthens the last tile's data wait.)
                for q in range(4):
                    nc.sync.dma_start(
                        out=x_t[:, 4 * q:4 * (q + 1), :],
                        in_=xt[j, :, 4 * q:4 * (q + 1), :],
                    )
                if j % 2 == 0:
                    w_pair = wout_pool.tile([P, 2, E], F32)
                ps = psum_pool.tile([P, E], F32)
                # Bias row seeds the accumulation: ps = ones.T @ gate_b.
                nc.tensor.matmul(ps[:], ones_sb[:], gb_sb[:], start=True, stop=False)
                for c in range(KC):
                    nc.tensor.matmul(
                        ps[:],
                        x_t[:, c, :],
                        wt_sb[:, c, :],
                        start=False,
                        stop=(c == KC - 1),
                    )
                e_t = sm_pool.tile([P, E], F32, tag="exp")
                ssum = stat_pool.tile([P, 1], F32, tag="sum")
                nc.scalar.activation(
                    e_t[:], ps[:],
                    mybir.ActivationFunctionType.Exp,
                    accum_out=ssum[:],
                )
                rcp = stat_pool.tile([P, 1], F32, tag="rcp")
                nc.vector.reciprocal(rcp[:], ssum[:])
                nc.vector.tensor_scalar_mul(w_pair[:, j % 2, :], e_t[:], rcp[:])
                # Top-2 on the unnormalized exps (same order as the softmax
                # weights: positive per-token scale) — runs without waiting
                # for the normalize; values then get the same e*rcp product.
                vals8 = top_pool.tile([P, 8], F32, tag="vals")
                idx8 = top_pool.tile([P, 8], U32, tag="idx")
                nc.vector.max(vals8[:], e_t[:])
                nc.vector.max_index(idx8[:], vals8[:], e_t[:])
                nc.vector.tensor_scalar_mul(tvi_t[:, j, 0:2], vals8[:, 0:2], rcp[:])
                nc.vector.tensor_copy(tvi_t[:, j, 2:4], idx8[:, 0:2])
                if j % 2 == 1:
                    # Mid-stream stores go through the otherwise-idle GpSimd
                    # SWDGE ring: ACT's FIFO would head-of-line block later
                    # exp dispatches, SP's would delay loads.
                    nc.gpsimd.dma_start(
                        out=w_out[:, j - 1:j + 1, :], in_=w_pair[:]
                    )
                    if j == NMT - 5:
                        # Tiles 0..11 of the packed top-2 are final by now;
                        # store them mid-stream so only the last 4 tiles'
                        # slice has to land after the compute tail.
                        nc.gpsimd.dma_start(
                            out=tvi_out.rearrange(
                                "p (j k) -> p j k", j=NMT
                            )[:, :NMT - 4, :],
                            in_=tvi_t[:, :NMT - 4, :],
                        )

            # SP's ring is idle by now; runs parallel to the last w store.
            nc.sync.dma_start(
                out=tvi_out.rearrange("p (j k) -> p j k", j=NMT)[:, NMT - 4:, :],
                in_=tvi_t[:, NMT - 4:, :],
            )

    nc.compile()
    return nc


def _get_program():
    if "nc" not in _cache:
        _cache["nc"] = _build_program()
    return _cache["nc"]


def _prep_inputs(x, gate_w, gate_b):
    x2d = np.ascontiguousarray(x, dtype=np.float32).reshape(TOK, D)
    # Pack gate weight: wtp[p, c, e] = gate_w[e, c*128 + p]
    wtp = np.ascontiguousarray(
        gate_w.T.reshape(KC, P, E).transpose(1, 0, 2)
    ).reshape(P, KC * E)
    gb = np.ascontiguousarray(gate_b, dtype=np.float32).reshape(1, E)
    in_maps = []
    for s in range(NCORES):
        sh = x2d[s * TPC:(s + 1) * TPC]
        # xp[j, p, c, t] = x[j*128 + t, c*128 + p]
        xp = np.ascontiguousarray(
            sh.reshape(NMT, P, KC, P).transpose(0, 3, 2, 1)
        )
        in_maps.append({"xt": xp, "wtp": wtp, "gb": gb})
    return in_maps


def kernel(x, gate_w, gate_b, _trace=False, _trace_kwargs=None):
    nc = _get_program()
    in_maps = _prep_inputs(x, gate_w, gate_b)
    res = run_bass_kernel_spmd(
        nc, in_maps, list(range(NCORES)), trace=_trace,
        **(_trace_kwargs or {}),
    )
    outs = res.results

    w_shards, tv_shards, ti_shards = [], [], []
    for s in range(NCORES):
        wdev = outs[s]["w_out"].reshape(P, NMT, E)
        w_shards.append(wdev.transpose(1, 0, 2).reshape(TPC, E))
        tvi = outs[s]["tvi_out"].reshape(P, NMT, 4)
        tv_shards.append(tvi[:, :, 0:2].transpose(1, 0, 2).reshape(TPC, 2))
        ti_shards.append(tvi[:, :, 2:4].transpose(1, 0, 2).reshape(TPC, 2))

    topk_w = np.concatenate(tv_shards, axis=0)
    topk_i = np.concatenate(ti_shards, axis=0)
    weights = np.concatenate(w_shards, axis=0)
    out = (
        topk_w.reshape(B, S, 2).astype(np.float32),
        np.rint(topk_i).reshape(B, S, 2).astype(np.int32),
        weights.reshape(B, S, E).astype(np.float32),
    )
    if _trace:
        return out, res
    return out


# revision 25
# speedup vs baseline: 1.0076x; 1.0001x over previous
"""MoE gating network (logits -> softmax -> top-2) on 8 trn2 NeuronCores.

Reference computation (jax):
    logits = einsum("bsd,ed->bse", x, gate_w) + gate_b     # [4, 4096, 64]
    weights = softmax(logits, axis=-1)
    topk_weights, topk_indices = top_k(weights, 2)
    return topk_weights, topk_indices, weights

Sharding: data parallel over tokens. 16384 tokens split into 8 shards of
2048; the tiny gate weight is replicated. Each core computes its shard's
logits on the PE (f32), softmax on ACT/DVE, and top-2 via the DVE
InstMax/InstMaxIndex ops (8 largest per partition, descending).

Per-core pipeline, 16 blocks of 128 tokens:
- x is host-packed to [block, partition, k-chunk, token] so each block's
  1 MB load is 128 descriptors of 8 KB (d_model lands on partitions, as
  the PE contraction requires, without any on-chip transpose).
- gate_b is folded into the PSUM accumulation as a K=1 matmul of a ones
  row against the bias row, so the ACT exp reads logits straight from
  PSUM (accum_out gives the softmax denominator in the same pass).
- top-2 values and indices are compacted per tile into one packed f32
  tile (indices as exact small floats) and stored once at the end.
- outputs use partition-major device layouts, unpermuted on the host.
"""

import numpy as np

import concourse.mybir as mybir
import concourse.tile as tile
from concourse import bacc
from concourse.bass_utils import run_bass_kernel_spmd

# Problem shape (hardcoded per contract; kernel.py must be self-contained).
B, S, D, E = 4, 4096, 2048, 64
NCORES = 8
TOK = B * S                  # 16384 tokens
TPC = TOK // NCORES          # 2048 tokens per core
P = 128                      # partitions
KC = D // P                  # 16 contraction chunks
NMT = TPC // P               # 16 token-tiles (= blocks) per core
F32 = mybir.dt.float32
U32 = mybir.dt.uint32

_cache = {}


def _build_program(with_bias):
    nc = bacc.Bacc(
        "TRN2", target_bir_lowering=False, debug=False, num_devices=NCORES
    )

    xt = nc.dram_tensor("xt", [NMT, P, KC, P], F32, kind="ExternalInput").ap()
    wtp = nc.dram_tensor("wtp", [P, KC * E], F32, kind="ExternalInput").ap()
    gb = None
    if with_bias:
        gb = nc.dram_tensor("gb", [1, E], F32, kind="ExternalInput").ap()
    # Outputs in partition-major layouts; host unpermutes.
    w_out = nc.dram_tensor("w_out", [P, NMT, E], F32, kind="ExternalOutput").ap()
    # Packed top-2: [p, j, 0:2] = values, [p, j, 2:4] = indices (as floats).
    tvi_out = nc.dram_tensor("tvi_out", [P, NMT * 4], F32, kind="ExternalOutput").ap()

    with tile.TileContext(nc) as tc:
        with (
            tc.tile_pool(name="const", bufs=1) as const_pool,
            tc.tile_pool(name="xin", bufs=8) as x_pool,
            tc.tile_pool(name="psum", bufs=8, space="PSUM") as psum_pool,
            tc.tile_pool(name="sm", bufs=6) as sm_pool,
            tc.tile_pool(name="stat", bufs=8) as stat_pool,
            tc.tile_pool(name="wout", bufs=3) as wout_pool,
            tc.tile_pool(name="top", bufs=4) as top_pool,
            tc.tile_pool(name="tvi", bufs=1) as tvi_pool,
        ):
            wt_sb = const_pool.tile([P, KC, E], F32)
            nc.sync.dma_start(out=wt_sb[:], in_=wtp.rearrange("p (c e) -> p c e", c=KC))
            if with_bias:
                gb_sb = const_pool.tile([1, E], F32)
                nc.sync.dma_start(out=gb_sb[:], in_=gb[:])
                ones_sb = const_pool.tile([1, P], F32)
                nc.vector.memset(ones_sb[:], 1.0)

            tvi_t = tvi_pool.tile([P, NMT, 4], F32)

            w_pair = None
            for j in range(NMT):
                x_t = x_pool.tile([P, KC, P], F32)
                # Quarter-loads: each 4-chunk group of contraction matmuls
                # only waits on its own quarter, so the PE trails the DMA
                # stream by ~1 quarter instead of a whole tile. (Both finer
                # and coarser splits measured slower: finer saturates the
                # exclusive HWDGE device at ~632 ns per dma_start; coarser
                # lengthens the last tile's data wait.)
                for q in range(4):
                    nc.sync.dma_start(
                        out=x_t[:, 4 * q:4 * (q + 1), :],
                        in_=xt[j, :, 4 * q:4 * (q + 1), :],
                    )
                if j % 2 == 0:
                    w_pair = wout_pool.tile([P, 2, E], F32)
                ps = psum_pool.tile([P, E], F32)
                if with_bias:
                    # Bias row seeds the accumulation: ps = ones.T @ gate_b.
                    nc.tensor.matmul(
                        ps[:], ones_sb[:], gb_sb[:], start=True, stop=False
                    )
                for c in range(KC):
                    nc.tensor.matmul(
                        ps[:],
                        x_t[:, c, :],
                        wt_sb[:, c, :],
                        start=(c == 0 and not with_bias),
                        stop=(c == KC - 1),
                    )
                e_t = sm_pool.tile([P, E], F32, tag="exp")
                ssum = stat_pool.tile([P, 1], F32, tag="sum")
                nc.scalar.activation(
                    e_t[:], ps[:],
                    mybir.ActivationFunctionType.Exp,
                    accum_out=ssum[:],
                )
                rcp = stat_pool.tile([P, 1], F32, tag="rcp")
                nc.vector.reciprocal(rcp[:], ssum[:])
                nc.vector.tensor_scalar_mul(w_pair[:, j % 2, :], e_t[:], rcp[:])
                # Top-2 on the unnormalized exps (same order as the softmax
                # weights: positive per-token scale) — runs without waiting
                # for the normalize; values then get the same e*rcp product.
                vals8 = top_pool.tile([P, 8], F32, tag="vals")
                idx8 = top_pool.tile([P, 8], U32, tag="idx")
                nc.vector.max(vals8[:], e_t[:])
                nc.vector.max_index(idx8[:], vals8[:], e_t[:])
                nc.vector.tensor_scalar_mul(tvi_t[:, j, 0:2], vals8[:, 0:2], rcp[:])
                nc.vector.tensor_copy(tvi_t[:, j, 2:4], idx8[:, 0:2])
                if j % 2 == 1:
                    # Mid-stream stores go through the otherwise-idle GpSimd
                    # SWDGE ring: ACT's FIFO would head-of-line block later
                    # exp dispatches, SP's would delay loads.
                    nc.gpsimd.dma_start(
                        out=w_out[:, j - 1:j + 1, :], in_=w_pair[:]
                    )
                    if j == NMT - 5:
                        # Tiles 0..11 of the packed top-2 are final by now;
                        # store them mid-stream so only the last 4 tiles'
                        # slice has to land after the compute tail.
                        nc.gpsimd.dma_start(
                            out=tvi_out.rearrange(
                                "p (j k) -> p j k", j=NMT
                            )[:, :NMT - 4, :],
                            in_=tvi_t[:, :NMT - 4, :],
                        )

            # SP's ring is idle by now; runs parallel to the last w store.
            nc.sync.dma_start(
                out=tvi_out.rearrange("p (j k) -> p j k", j=NMT)[:, NMT - 4:, :],
                in_=tvi_t[:, NMT - 4:, :],
            )

    nc.compile()
    return nc


def _get_program(with_bias=False):
    key = ("bias", with_bias)
    if key not in _cache:
        _cache[key] = _build_program(with_bias)
    return _cache[key]


def _prep_inputs(x, gate_w, gate_b, with_bias):
    x2d = np.ascontiguousarray(x, dtype=np.float32).reshape(TOK, D)
    # Pack gate weight: wtp[p, c, e] = gate_w[e, c*128 + p]
    wtp = np.ascontiguousarray(
        gate_w.T.reshape(KC, P, E).transpose(1, 0, 2)
    ).reshape(P, KC * E)
    in_maps = []
    for s in range(NCORES):
        sh = x2d[s * TPC:(s + 1) * TPC]
        # xp[j, p, c, t] = x[j*128 + t, c*128 + p]
        xp = np.ascontiguousarray(
            sh.reshape(NMT, P, KC, P).transpose(0, 3, 2, 1)
        )
        in_maps.append({"xt": xp, "wtp": wtp})
        if with_bias:
            in_maps[-1]["gb"] = np.ascontiguousarray(
                gate_b, dtype=np.float32
            ).reshape(1, E)
    return in_maps


def kernel(x, gate_w, gate_b, _trace=False, _trace_kwargs=None):
    # gate_b is all-zeros in this problem's setup; the zero path skips the
    # bias-seeding matmul and its const loads. Nonzero bias still works via
    # the general program variant.
    with_bias = bool(np.any(np.asarray(gate_b)))
    nc = _get_program(with_bias)
    in_maps = _prep_inputs(x, gate_w, gate_b, with_bias)
    res = run_bass_kernel_spmd(
        nc, in_maps, list(range(NCORES)), trace=_trace,
        **(_trace_kwargs or {}),
    )
    outs = res.results

    w_shards, tv_shards, ti_shards = [], [], []
    for s in range(NCORES):
        wdev = outs[s]["w_out"].reshape(P, NMT, E)
        w_shards.append(wdev.transpose(1, 0, 2).reshape(TPC, E))
        tvi = outs[s]["tvi_out"].reshape(P, NMT, 4)
        tv_shards.append(tvi[:, :, 0:2].transpose(1, 0, 2).reshape(TPC, 2))
        ti_shards.append(tvi[:, :, 2:4].transpose(1, 0, 2).reshape(TPC, 2))

    topk_w = np.concatenate(tv_shards, axis=0)
    topk_i = np.concatenate(ti_shards, axis=0)
    weights = np.concatenate(w_shards, axis=0)
    out = (
        topk_w.reshape(B, S, 2).astype(np.float32),
        np.rint(topk_i).reshape(B, S, 2).astype(np.int32),
        weights.reshape(B, S, E).astype(np.float32),
    )
    if _trace:
        return out, res
    return out
